# revision 6
# baseline (speedup 1.0000x reference)
"""Distributed causal multi-head attention for TRN2 (8 NeuronCores).

Sharding: tensor-parallel over heads (2 heads/core) for QKV projection and
attention; AllToAll (split in two column halves) switches to
sequence-sharding for the output projection (each core emits 512 rows of
the final output, stitched on host).

Key structure (v3):
  - x is transposed on the HOST: the kernel receives xT [D, S] and loads
    it with straight 1KB-packet DMAs (the crossbar transpose DMAs moved
    8MB in 2-byte packets and stalled everything).
  - wq/wk/wv are host-shuffled to the [128, 8*128] partition layout so
    their DMAs are straight 2KB-per-partition copies.
  - V is projected directly into its [skv, hk] attention layout by using
    xT tiles as the stationary operand and wv as the moving operand.
  - proj(t) and attn(t) issue in one fused loop; the Tile scheduler fills
    TensorE gaps during the ScalarE-bound attention (HAM stays warm).
  - Scores for both heads land in one contiguous PSUM tile [128, 2*512]:
    a single exp instruction covers both heads.
  - The causal mask is applied by an accumulating matmul (identity as
    stationary operand, mask as moving operand) - no DVE hop between the
    score matmul and the exp.
  - Attention inner loop is software-pipelined: scores/exp run 1-2 chunks
    ahead of the AV matmuls so ScalarE never waits on TensorE.
  - The AV accumulator (PSUM) is copied to SBUF at tile end so the
    softmax divide chain (reciprocal is ~3.3us on DVE) stays off the
    TensorE critical path.
  - AllToAll is split into two column halves; the output projection of
    the first half overlaps the second collective.

Layout discipline (contraction dim must sit on SBUF partitions):
  - xT tiles    [d, s]     : straight DMA from host-transposed x (bf16)
  - Q^T, K^T    [hk, s]    : projection matmuls, lhsT=w (bf16)
  - V_aug       [skv, 65]  : per skv-chunk, per head, bf16; col 64 = 1.0
                             (softmax denominator accumulates in AV row 64)
  - S^T tiles   [skv, 2*sq]: PSUM f32; exp on ScalarE -> P^T bf16 in SBUF
  - vals^T      [hk, sq]   : AV accumulation / denom divide, bf16, A2A'd
"""

import sys

sys.path.insert(0, "/opt/trn_rl_repo")

import ml_dtypes
import numpy as np

from concourse import bacc, bass, mybir, tile
from concourse.bass_utils import run_bass_kernel_spmd

S, D, H, K = 4096, 1024, 16, 64
NCORES = 8
HPC = H // NCORES          # heads per core (2)
HKC = HPC * K              # local head*dim columns (128)
SQ = S // NCORES           # seq rows owned per core (512)
SQT = 512                  # sq tile width in attention
NCH = S // 128             # total skv chunks (32)
SQH = SQ // 2              # a2a column half (256)
F32 = mybir.dt.float32
BF16 = mybir.dt.bfloat16
EXP = mybir.ActivationFunctionType.Exp
BF16NP = ml_dtypes.bfloat16

_CACHE: dict = {}


def _build(causal: bool):
    nc = bacc.Bacc(
        "TRN2", target_bir_lowering=False, debug=False, num_devices=NCORES
    )
    cores = list(range(NCORES))

    xT_full = nc.dram_tensor("xT_full", [D, S], BF16, kind="ExternalInput")
    # host-shuffled: row p holds w[a*128+p, :] for a=0..7 concatenated
    wq_c = nc.dram_tensor("wq_c", [128, 8 * HKC], BF16, kind="ExternalInput")
    wk_c = nc.dram_tensor("wk_c", [128, 8 * HKC], BF16, kind="ExternalInput")
    wv_c = nc.dram_tensor("wv_c", [128, 8 * HKC], BF16, kind="ExternalInput")
    wo_f = nc.dram_tensor("wo_f", [H * K, D], BF16, kind="ExternalInput")
    bq_c = nc.dram_tensor("bq_c", [HKC, 1], F32, kind="ExternalInput")
    bk_c = nc.dram_tensor("bk_c", [HKC, 1], F32, kind="ExternalInput")
    bv_r = nc.dram_tensor("bv_r", [1, HKC], F32, kind="ExternalInput")
    bo_r = nc.dram_tensor("bo_r", [1, D], F32, kind="ExternalInput")
    masks = nc.dram_tensor("masks", [128, 128], BF16, kind="ExternalInput")
    ident = nc.dram_tensor("ident", [128, 128], BF16, kind="ExternalInput")
    out_t = nc.dram_tensor("out", [SQ, D], F32, kind="ExternalOutput")

    with tile.TileContext(nc) as tc:
        with tc.tile_pool(name="dram", bufs=1, space="DRAM") as dpool:
            a2a_in = [
                dpool.tile([NCORES * HKC, SQH], BF16, name=f"a2a_in{i}")
                for i in range(2)
            ]
            a2a_out = [
                dpool.tile([NCORES * HKC, SQH], BF16, name=f"a2a_out{i}")
                for i in range(2)
            ]

            with tc.tile_pool(name="persist", bufs=1) as pp:
                # ---- persistent SBUF ----
                wq_sb = pp.tile([128, 8, HKC], BF16, name="wq_sb")
                wk_sb = pp.tile([128, 8, HKC], BF16, name="wk_sb")
                wv_sb = pp.tile([128, 8, HKC], BF16, name="wv_sb")
                nc.sync.dma_start(
                    out=wq_sb, in_=wq_c.ap().rearrange("p (a h) -> p a h", a=8)
                )
                nc.sync.dma_start(
                    out=wk_sb, in_=wk_c.ap().rearrange("p (a h) -> p a h", a=8)
                )
                nc.scalar.dma_start(
                    out=wv_sb, in_=wv_c.ap().rearrange("p (a h) -> p a h", a=8)
                )
                bq_sb = pp.tile([128, 1], F32, name="bq_sb")
                bk_sb = pp.tile([128, 1], F32, name="bk_sb")
                for bsb, bdr in ((bq_sb, bq_c), (bk_sb, bk_c)):
                    nc.scalar.dma_start(out=bsb, in_=bdr.ap())
                bv_row = pp.tile([1, HKC], F32, name="bv_row")
                nc.scalar.dma_start(out=bv_row, in_=bv_r.ap())
                bv_bc = pp.tile([128, HKC], F32, name="bv_bc")
                nc.gpsimd.partition_broadcast(bv_bc, bv_row)
                masks_sb = pp.tile([128, 128], BF16, name="masks_sb")
                nc.scalar.dma_start(out=masks_sb, in_=masks.ap())
                ident_sb = pp.tile([128, 128], BF16, name="ident_sb")
                nc.scalar.dma_start(out=ident_sb, in_=ident.ap())
                nbias = pp.tile([128, 1], F32, name="nbias")
                nc.vector.memset(nbias, -3.0)
                qT_sb = pp.tile([128, S], BF16, name="qT_sb")
                kT_sb = pp.tile([128, S], BF16, name="kT_sb")
                v_aug = pp.tile([128, NCH, HPC, K + 1], BF16, name="v_aug")
                nc.vector.memset(v_aug, 1.0)  # presets the ones columns

                with tc.tile_pool(name="xtp", bufs=3) as xtp, tc.tile_pool(
                    name="pj", bufs=2, space="PSUM"
                ) as pj, tc.tile_pool(
                    name="ps", bufs=2, space="PSUM"
                ) as psp, tc.tile_pool(
                    name="pv", bufs=2, space="PSUM"
                ) as pvp, tc.tile_pool(
                    name="pT", bufs=3
                ) as pTp, tc.tile_pool(
                    name="vc", bufs=3
                ) as vcp, tc.tile_pool(
                    name="sm", bufs=4
                ) as smp, tc.tile_pool(name="vs", bufs=4) as vsp:

                    def proj_tile(t):
                        """QKV projection for x tile t (512 rows)."""
                        xt = xtp.tile([128, 8, SQT], BF16, name="xt")
                        nc.sync.dma_start(
                            out=xt,
                            in_=xT_full.ap()[
                                :, t * SQT : (t + 1) * SQT
                            ].rearrange("(a p) s -> p a s", p=128),
                        )
                        for wsb, bsb, dst in (
                            (wq_sb, bq_sb, qT_sb),
                            (wk_sb, bk_sb, kT_sb),
                        ):
                            pqk = pj.tile([128, SQT], F32, name="pj")
                            for dc in range(8):
                                nc.tensor.matmul(
                                    pqk,
                                    lhsT=wsb[:, dc, :],
                                    rhs=xt[:, dc, :],
                                    start=(dc == 0),
                                    stop=(dc == 7),
                                )
                            nc.vector.tensor_scalar_add(
                                out=dst[:, t * SQT : (t + 1) * SQT],
                                in0=pqk,
                                scalar1=bsb,
                            )
                        # V directly in [skv, hk] layout: xT slice stationary
                        for j in range(4):
                            ch = 4 * t + j
                            pvs = pj.tile([128, HKC], F32, name="pj")
                            for dc in range(8):
                                nc.tensor.matmul(
                                    pvs,
                                    lhsT=xt[:, dc, j * 128 : (j + 1) * 128],
                                    rhs=wv_sb[:, dc, :],
                                    start=(dc == 0),
                                    stop=(dc == 7),
                                )
                            for h in range(HPC):
                                nc.vector.tensor_add(
                                    out=v_aug[:, ch, h, 0:K],
                                    in0=pvs[:, h * K : (h + 1) * K],
                                    in1=bv_bc[:, h * K : (h + 1) * K],
                                )

                    def score_chunk(t, ch):
                        """Scores+exp for chunk ch of tile t. Returns
                        (pT, off) for the later AV matmuls."""
                        jm = ch - 4 * t
                        diag = causal and 0 <= jm < 4
                        off = jm * 128 if (causal and 0 < jm < 4) else 0
                        ps = psp.tile([128, HPC * SQT], F32, name="ps")
                        for h in range(HPC):
                            hs = h * K
                            nc.tensor.matmul(
                                ps[:, h * SQT + off : (h + 1) * SQT],
                                lhsT=kT_sb[
                                    hs : hs + K, ch * 128 : (ch + 1) * 128
                                ],
                                rhs=qT_sb[
                                    hs : hs + K, t * SQT + off : (t + 1) * SQT
                                ],
                                start=True,
                                stop=not diag,
                            )
                            if diag:
                                # causal mask via accumulating matmul:
                                # ident.T @ masks == masks, added in-place
                                mo = h * SQT + jm * 128
                                nc.tensor.matmul(
                                    ps[:, mo : mo + 128],
                                    lhsT=ident_sb,
                                    rhs=masks_sb,
                                    start=False,
                                    stop=True,
                                )
                        pT = pTp.tile([128, HPC * SQT], BF16, name="pT")
                        # one exp covers both heads; for diagonal chunks
                        # the [512, 512+off) middle region is stale PSUM
                        # (never read downstream)
                        nc.scalar.activation(
                            out=pT[:, off : HPC * SQT],
                            in_=ps[:, off : HPC * SQT],
                            func=EXP,
                            scale=0.125,
                            bias=nbias,
                        )
                        return pT, off

                    def av_chunk(t, ch, nchunks, pv, pT, off):
                        for h in range(HPC):
                            nc.tensor.matmul(
                                pv[h][:, off:SQT],
                                lhsT=v_aug[:, ch, h, :],
                                rhs=pT[:, h * SQT + off : (h + 1) * SQT],
                                start=(ch == 0),
                                stop=(ch == nchunks - 1),
                            )

                    def attn_tile(t):
                        """Causal attention for sq tile t, both heads.
                        Scores/exp run 2 chunks ahead of AV."""
                        nchunks = 4 * (t + 1) if causal else NCH
                        pv = [
                            pvp.tile([K + 1, SQT], F32, name="pv")
                            for _ in range(HPC)
                        ]
                        pending = []
                        for ch in range(nchunks):
                            pending.append((ch, *score_chunk(t, ch)))
                            if len(pending) > 1:
                                c0, pT0, off0 = pending.pop(0)
                                av_chunk(t, c0, nchunks, pv, pT0, off0)
                        for c0, pT0, off0 in pending:
                            av_chunk(t, c0, nchunks, pv, pT0, off0)
                        for h in range(HPC):
                            hs = h * K
                            # copy the AV accumulator out of PSUM so the
                            # divide chain doesn't hold the PSUM slot
                            vcpt = vcp.tile([K + 1, SQT], F32, name="vcp")
                            nc.vector.tensor_copy(out=vcpt, in_=pv[h])
                            recip = smp.tile([1, SQT], F32, name="recip")
                            nc.vector.reciprocal(
                                out=recip, in_=vcpt[K : K + 1, :]
                            )
                            bcn = smp.tile([K, SQT], F32, name="bcn")
                            nc.gpsimd.partition_broadcast(bcn, recip)
                            vst = vsp.tile([K, SQT], BF16, name="vst")
                            nc.vector.tensor_mul(
                                out=vst, in0=vcpt[0:K, :], in1=bcn
                            )
                            for i in range(2):
                                nc.sync.dma_start(
                                    out=a2a_in[i][
                                        t * HKC + hs : t * HKC + hs + K, :
                                    ],
                                    in_=vst[:, i * SQH : (i + 1) * SQH],
                                )

                    for t in range(8):
                        proj_tile(t)
                        attn_tile(t)

                    # loads needed only for P3: issue late so they don't
                    # compete with the pipeline's DMAs
                    wo_sb = pp.tile([128, 8, D], BF16, name="wo_sb")
                    nc.scalar.dma_start(
                        out=wo_sb,
                        in_=wo_f.ap().rearrange("(a p) d -> p a d", p=128),
                    )
                    bo_sb = pp.tile([1, D], F32, name="bo_sb")
                    nc.scalar.dma_start(out=bo_sb, in_=bo_r.ap())
                    bo_bc = pp.tile([128, D], F32, name="bo_bc")
                    nc.gpsimd.partition_broadcast(bo_bc, bo_sb)

                    # ---- P3: output projection, pipelined per a2a half --
                    with tc.tile_pool(name="op", bufs=2) as op, tc.tile_pool(
                        name="ob", bufs=2
                    ) as obp:
                        for half in range(2):
                            nc.gpsimd.collective_compute(
                                "AllToAll",
                                mybir.AluOpType.bypass,
                                replica_groups=[cores],
                                ins=[a2a_in[half].opt()],
                                outs=[a2a_out[half].opt()],
                            )
                        for half in range(2):
                            va_sb = op.tile([128, 8, SQH], BF16, name="va")
                            for a in range(8):
                                nc.sync.dma_start(
                                    out=va_sb[:, a, :],
                                    in_=a2a_out[half][
                                        a * 128 : (a + 1) * 128, :
                                    ],
                                )
                            for mm in range(2):
                                m = half * 2 + mm
                                o_sb = obp.tile([128, D], F32, name="o_sb")
                                for dh in range(2):
                                    po = pj.tile([128, SQT], F32, name="pj")
                                    for hkc in range(8):
                                        nc.tensor.matmul(
                                            po,
                                            lhsT=va_sb[
                                                :,
                                                hkc,
                                                mm * 128 : (mm + 1) * 128,
                                            ],
                                            rhs=wo_sb[
                                                :,
                                                hkc,
                                                dh * 512 : (dh + 1) * 512,
                                            ],
                                            start=(hkc == 0),
                                            stop=(hkc == 7),
                                        )
                                    nc.vector.tensor_add(
                                        out=o_sb[:, dh * 512 : (dh + 1) * 512],
                                        in0=po,
                                        in1=bo_bc[:, dh * 512 : (dh + 1) * 512],
                                    )
                                nc.sync.dma_start(
                                    out=out_t.ap()[
                                        m * 128 : (m + 1) * 128, :
                                    ],
                                    in_=o_sb,
                                )

    nc.compile()
    return nc


def _get_nc(causal: bool):
    if causal not in _CACHE:
        _CACHE[causal] = _build(causal)
    return _CACHE[causal]


def _shuffle_w(w):
    # [D, HKC] -> [128, 8*HKC]: row p holds w[a*128+p, :] for a=0..7
    return np.ascontiguousarray(
        w.reshape(8, 128, HKC).transpose(1, 0, 2).reshape(128, 8 * HKC)
    )


def _make_in_maps(x, wq, bq, wk, bk, wv, bv, wo, bo):
    xT = np.ascontiguousarray(
        np.asarray(x, np.float32).reshape(S, D).T.astype(BF16NP)
    )
    wqf = np.asarray(wq, np.float32).reshape(D, H * K).astype(BF16NP)
    wkf = np.asarray(wk, np.float32).reshape(D, H * K).astype(BF16NP)
    wvf = np.asarray(wv, np.float32).reshape(D, H * K).astype(BF16NP)
    wof = np.ascontiguousarray(
        np.asarray(wo, np.float32).reshape(H * K, D).astype(BF16NP)
    )
    bqf = np.asarray(bq, np.float32).reshape(H * K)
    bkf = np.asarray(bk, np.float32).reshape(H * K)
    bvf = np.asarray(bv, np.float32).reshape(H * K)
    bof = np.ascontiguousarray(np.asarray(bo, np.float32).reshape(1, D))

    p = np.arange(128)[:, None]
    c = np.arange(128)[None, :]
    mask_np = np.where(c >= p, 0.0, -1e9).astype(BF16NP)
    ident_np = np.eye(128, dtype=np.float32).astype(BF16NP)

    in_maps = []
    for core in range(NCORES):
        hk0 = core * HKC
        in_maps.append(
            {
                "xT_full": xT,
                "wq_c": _shuffle_w(wqf[:, hk0 : hk0 + HKC]),
                "wk_c": _shuffle_w(wkf[:, hk0 : hk0 + HKC]),
                "wv_c": _shuffle_w(wvf[:, hk0 : hk0 + HKC]),
                "wo_f": wof,
                "bq_c": np.ascontiguousarray(
                    bqf[hk0 : hk0 + HKC].reshape(HKC, 1)
                ),
                "bk_c": np.ascontiguousarray(
                    bkf[hk0 : hk0 + HKC].reshape(HKC, 1)
                ),
                "bv_r": np.ascontiguousarray(
                    bvf[hk0 : hk0 + HKC].reshape(1, HKC)
                ),
                "bo_r": bof,
                "masks": mask_np,
                "ident": ident_np,
            }
        )
    return in_maps


def _run(inputs: dict, trace: bool = False):
    causal = bool(int(np.asarray(inputs["is_causal"])))
    nc = _get_nc(causal)
    in_maps = _make_in_maps(
        inputs["x"], inputs["wq"], inputs["bq"], inputs["wk"], inputs["bk"],
        inputs["wv"], inputs["bv"], inputs["wo"], inputs["bo"],
    )
    res = run_bass_kernel_spmd(
        nc, in_maps, list(range(NCORES)), trace=trace
    )
    out = np.empty((1, S, D), np.float32)
    for core in range(NCORES):
        out[0, core * SQ : (core + 1) * SQ] = res.results[core]["out"]
    return out, res


def kernel(**inputs) -> np.ndarray:
    out, _ = _run(inputs, trace=False)
    return out
